# revision 14
# baseline (speedup 1.0000x reference)
"""GNN message-passing layer (ConvLayer) on 8 Trainium2 NeuronCores.

Strategy (full_io contract):
  * Host: sort edges by idx1, partition NODES into 8 contiguous ranges with
    balanced edge counts; each core gets all edges of its node range, so
    segment sums never cross cores.
  * Edges per core are chunked into groups of 512 whose idx1 values span a
    window of <=128 consecutive nodes (ghost-edge padding keeps the invariant
    and makes all cores' chunk counts equal -> one SPMD program).
  * Device phase E1 (per 512-edge chunk, feature-major activations
    [128 feat x 512 edge]):
       - window trick:   n1 contribution  = (window @ We1a)^T @ OHn
       - idx2 gather:    indirect DMA of node rows, PE-transpose to f-major
       - MLP with lrelu as 0.2x + 0.8relu(x) folded into doubled matmuls
       - y (pre-BN MLP out) -> DRAM; per-channel sum/sumsq via ACT accum_out
       - segment sums: OHe^T-style matmul -> per-chunk [128 node x 128 feat]
         staging tile in DRAM (combined later by indirect gather)
  * AllReduce #1: edge-BN sum/sumsq (ghost contributions removed via
    host-computed constants).  E2: edge_new = edge_fea + s*y + t.
  * Phase V: per 128-node tile gather the <=NWIN covering chunk staging tiles,
    add, scale by mask/cnt, PE-transpose (scaled-identity trick), edge-BN
    affine + t x mask outer product -> vi_e_bar^T; node MLP; AllReduce #2 for
    node-BN stats; node_new = node_fea + sv*y2 + tv.
  * Host: inverse-permute edge output, concatenate node ranges.
"""

import math
import os
import numpy as np

import concourse.bacc as bacc
import concourse.bass as bass
import concourse.mybir as mybir
import concourse.tile as tile
from concourse.bass import IndirectOffsetOnAxis
from concourse.bass_utils import run_bass_kernel_spmd
from concourse.masks import make_identity

P = 128
CHUNK = 512
SUB = CHUNK // P  # 4 subtiles per chunk
NCORES = 8
F32 = mybir.dt.float32
BF16 = mybir.dt.bfloat16
I32 = mybir.dt.int32
BN_EPS = 1e-5
LRELU_SLOPE = 0.2

AX = mybir.AxisListType
AF = mybir.ActivationFunctionType
ALU = mybir.AluOpType

# rows in the "consts" [NC_ROWS, 128] tensor
C_BE1, C_BE2, C_BE3, C_BV1, C_BV2, C_BV3 = 0, 1, 2, 3, 4, 5
C_GE, C_BETAE, C_GV, C_BETAV = 6, 7, 8, 9
C_SCORR_E, C_QCORR_E, C_INVE = 10, 11, 12
C_SCORR_V, C_QCORR_V, C_INVN = 13, 14, 15
C_EPS = 16
NC_ROWS = 17


def _lrelu_np(x):
    return np.where(x > 0, x, LRELU_SLOPE * x)


# --------------------------------------------------------------------------
# host preparation
# --------------------------------------------------------------------------

class _Cfg:
    pass


def _host_prep(node_fea, edge_fea, idx1, idx2, W):
    N, D = node_fea.shape
    E = idx1.shape[0]
    assert D == P

    cfg = _Cfg()
    cfg.N, cfg.E = N, E

    order = np.argsort(idx1, kind="stable")
    idx1s = idx1[order]
    idx2s = idx2[order]
    cnt = np.bincount(idx1, minlength=N).astype(np.int64)
    cum = np.concatenate([[0], np.cumsum(cnt)])  # cum[n] = #edges with idx1 < n

    # node boundaries -> balanced edge counts
    nb = [0]
    for k in range(1, NCORES):
        t = int(round(k * E / NCORES))
        n = int(np.searchsorted(cum, t, side="left"))
        nb.append(min(max(n, nb[-1]), N))
    nb.append(N)
    cfg.nb = nb

    # per-core chunking
    per_core = []
    max_chunks = 0
    for k in range(NCORES):
        n0, n1 = nb[k], nb[k + 1]
        e0, e1 = int(cum[n0]), int(cum[n1])
        i1 = idx1s[e0:e1]
        chunks = []  # (start_in_core, take, Wabs)
        s = 0
        while s < len(i1):
            Wabs = int(i1[s])
            lim = int(np.searchsorted(i1, Wabs + P, side="left"))
            take = min(CHUNK, lim - s)
            chunks.append((s, take, Wabs))
            s += take
        per_core.append((n0, n1, e0, e1, chunks))
        max_chunks = max(max_chunks, len(chunks))

    n_chunks = max_chunks
    E_pad = n_chunks * CHUNK
    maxNloc = max(n1 - n0 for (n0, n1, _, _, _) in per_core)
    N_pad = int(math.ceil((maxNloc + P) / CHUNK) * CHUNK)
    cfg.n_chunks, cfg.E_pad, cfg.N_pad = n_chunks, E_pad, N_pad
    GHOST_W = 1 << 30

    # build per-core arrays
    ZR = n_chunks * P  # index of first zero row in segstage
    import ml_dtypes
    nfpad = np.vstack([node_fea, np.zeros((1, P), np.float32)]).astype(
        ml_dtypes.bfloat16)  # gather table (bf16)
    cores = []
    nwin = 1
    for k in range(NCORES):
        n0, n1, e0, e1, chunks = per_core[k]
        N_loc = n1 - n0
        ids = order[e0:e1]
        i1 = idx1s[e0:e1]
        i2 = idx2s[e0:e1]

        eid = np.full(E_pad, -1, np.int64)
        i1rel = np.full(E_pad, -1e9, np.float32)
        i2flat = np.full(E_pad, N, np.int64)
        Wlist = np.full(n_chunks, GHOST_W, np.int64)
        for t, (s, take, Wabs) in enumerate(chunks):
            dst = t * CHUNK
            eid[dst:dst + take] = ids[s:s + take]
            i1rel[dst:dst + take] = (i1[s:s + take] - Wabs).astype(np.float32)
            i2flat[dst:dst + take] = i2[s:s + take]
            Wlist[t] = Wabs

        valid = eid >= 0
        eft = np.zeros((P, E_pad), np.float32)
        eft[:, valid] = edge_fea[eid[valid]].T

        i2p = (i2flat.reshape(n_chunks, SUB, P).transpose(0, 2, 1)
               .reshape(n_chunks * P, SUB).astype(np.int32))

        wins = np.zeros((P, n_chunks * P), np.float32)
        for t in range(n_chunks):
            Wabs = Wlist[t]
            if Wabs >= GHOST_W:
                continue
            hi = min(Wabs + P, N)
            wins[:, t * P: t * P + (hi - Wabs)] = node_fea[Wabs:hi].T

        # gidx: for each local node, the covering chunk staging rows
        slots = [[] for _ in range(N_pad)]
        for t in range(n_chunks):
            Wabs = Wlist[t]
            if Wabs >= GHOST_W:
                continue
            lo = max(Wabs, n0)
            hi = min(Wabs + P, n1)
            for n in range(lo, hi):
                slots[n - n0].append(t * P + (n - Wabs))
        nwin = max(nwin, max((len(s) for s in slots), default=1))
        cores.append(dict(
            k=k, n0=n0, n1=n1, N_loc=N_loc, eid=eid, eft=eft, i2p=i2p,
            wins=wins, i1rel=i1rel, slots=slots,
        ))
    cfg.nwin = nwin

    for c in cores:
        gidx = np.full((N_pad, nwin), ZR, np.int32)
        for r, s in enumerate(c["slots"]):
            for j, v in enumerate(s):
                gidx[r, j] = v
        c["gidx"] = gidx
        n0, n1 = c["n0"], c["n1"]
        ccore = cnt[n0:n1]
        acol = np.zeros(N_pad, np.float32)
        mrow = np.zeros(N_pad, np.float32)
        acol[:n1 - n0] = (ccore > 0) / np.maximum(ccore, 1)
        mrow[:n1 - n0] = (ccore > 0).astype(np.float32)
        c["acol"] = acol
        c["mrow"] = mrow
        nftc = np.zeros((P, N_pad), np.float32)
        nftc[:, :n1 - n0] = node_fea[n0:n1].T
        c["nftc"] = nftc
        c["nftc_h"] = nftc.astype(ml_dtypes.bfloat16)
        c["eft_h"] = c["eft"].astype(ml_dtypes.bfloat16)
        c["wins_h"] = c["wins"].astype(ml_dtypes.bfloat16)
        c["i1rel_h"] = c["i1rel"].astype(ml_dtypes.bfloat16)

    # weights / consts ---------------------------------------------------
    s, r = LRELU_SLOPE, 1.0 - LRELU_SLOPE
    wcols = [
        W["We1"][0:P], W["We1"][P:2 * P], W["We1"][2 * P:3 * P],
        s * W["We2"], r * W["We2"], s * W["We3"], r * W["We3"],
        W["Wv1"][0:P], W["Wv1"][P:2 * P],
        s * W["Wv2"], r * W["Wv2"], s * W["Wv3"], r * W["Wv3"],
    ]
    cfg.w_off = {}
    names = ["e1a", "e1b", "e1c", "e2a", "e2r", "e3a", "e3r",
             "v1a", "v1b", "v2a", "v2r", "v3a", "v3r"]
    for i, nm in enumerate(names):
        cfg.w_off[nm] = i * P
    wsplit = np.concatenate(wcols, axis=0).astype(np.float32).reshape(
        len(wcols), P, P).transpose(1, 0, 2).reshape(P, len(wcols) * P)
    wsplit = wsplit.astype(ml_dtypes.bfloat16)
    # wsplit[p, i*128 + c] = wcols[i][p, c]  (each block a [in x out] lhsT)

    # ghost-output constants
    h = _lrelu_np(W["be1"]); h = _lrelu_np(h @ W["We2"] + W["be2"])
    c_e = h @ W["We3"] + W["be3"]
    h = _lrelu_np(W["bv1"]); h = _lrelu_np(h @ W["Wv2"] + W["bv2"])
    c_v = h @ W["Wv3"] + W["bv3"]
    G_e = NCORES * E_pad - E
    G_v = NCORES * N_pad - N

    consts = np.zeros((NC_ROWS, P), np.float32)
    consts[C_BE1], consts[C_BE2], consts[C_BE3] = W["be1"], W["be2"], W["be3"]
    consts[C_BV1], consts[C_BV2], consts[C_BV3] = W["bv1"], W["bv2"], W["bv3"]
    consts[C_GE], consts[C_BETAE] = W["gamma_e"], W["beta_e"]
    consts[C_GV], consts[C_BETAV] = W["gamma_v"], W["beta_v"]
    consts[C_SCORR_E], consts[C_QCORR_E] = G_e * c_e, G_e * c_e * c_e
    consts[C_INVE] = 1.0 / E
    consts[C_SCORR_V], consts[C_QCORR_V] = G_v * c_v, G_v * c_v * c_v
    consts[C_INVN] = 1.0 / N
    consts[C_EPS] = BN_EPS

    cfg.in_maps = []
    for c in cores:
        cfg.in_maps.append({
            "eft": c["eft"], "eft_h": c["eft_h"], "wins": c["wins_h"],
            "i1rel": c["i1rel_h"], "i2p": c["i2p"], "nfpad": nfpad,
            "nftc": c["nftc"], "nftc_h": c["nftc_h"], "gidx": c["gidx"],
            "acol": c["acol"], "mrow": c["mrow"],
            "wsplit": wsplit, "consts": consts,
        })
    cfg.cores = cores
    return cfg


# --------------------------------------------------------------------------
# device program
# --------------------------------------------------------------------------

def build_program(N, n_chunks, N_pad, nwin, w_off, debug=False, dbg_taps=False):
    E_pad = n_chunks * CHUNK
    NV = N_pad // CHUNK
    ZR_ROWS = (n_chunks + 1) * P

    nc = bacc.Bacc("TRN2", target_bir_lowering=False, debug=debug,
                   num_devices=NCORES)

    def din(name, shape, dt=F32):
        return nc.dram_tensor(name, shape, dt, kind="ExternalInput").ap()

    eft = din("eft", [P, E_pad])
    eft_h = din("eft_h", [P, E_pad], BF16)
    wins = din("wins", [P, n_chunks * P], BF16)
    i1rel = din("i1rel", [E_pad], BF16)
    i2p = din("i2p", [n_chunks * P, SUB], I32)
    nfpad = din("nfpad", [N + 1, P], BF16)
    nftc = din("nftc", [P, N_pad])
    nftc_h = din("nftc_h", [P, N_pad], BF16)
    gidx = din("gidx", [N_pad, nwin], I32)
    acol = din("acol", [N_pad])
    mrow = din("mrow", [N_pad])
    wsplit = din("wsplit", [P, 13 * P], BF16)
    consts = din("consts", [NC_ROWS, P])

    edge_newt = nc.dram_tensor("edge_newt", [P, E_pad], F32,
                               kind="ExternalOutput").ap()
    node_newt = nc.dram_tensor("node_newt", [P, N_pad], F32,
                               kind="ExternalOutput").ap()
    if dbg_taps:
        d_yt = nc.dram_tensor("d_yt", [P, n_chunks * CHUNK], F32,
                              kind="ExternalOutput").ap()
        d_seg = nc.dram_tensor("d_seg", [(n_chunks + 1) * P, P], F32,
                               kind="ExternalOutput").ap()
        d_st1 = nc.dram_tensor("d_st1", [P, 2], F32,
                               kind="ExternalOutput").ap()
        d_ar1 = nc.dram_tensor("d_ar1", [2, P], F32,
                               kind="ExternalOutput").ap()
        d_secol = nc.dram_tensor("d_secol", [P, 2], F32,
                                 kind="ExternalOutput").ap()
        d_vi = nc.dram_tensor("d_vi", [P, N_pad], F32,
                              kind="ExternalOutput").ap()
        d_y2 = nc.dram_tensor("d_y2", [P, N_pad], F32,
                              kind="ExternalOutput").ap()

    rg = [list(range(NCORES))]

    from contextlib import ExitStack
    with tile.TileContext(nc) as tc, ExitStack() as _es:
        cpool = _es.enter_context(tc.tile_pool(name="cpool", bufs=1))
        sb = _es.enter_context(tc.tile_pool(name="sb", bufs=2))
        sb2 = _es.enter_context(tc.tile_pool(name="sb2", bufs=2))
        psT = _es.enter_context(tc.tile_pool(name="psT", bufs=2, space="PSUM"))
        psM = _es.enter_context(tc.tile_pool(name="psM", bufs=3, space="PSUM"))
        psS = _es.enter_context(tc.tile_pool(name="psS", bufs=2, space="PSUM"))
        dpool = _es.enter_context(tc.tile_pool(name="dpool", bufs=1,
                                               space="DRAM"))
        yt_d = dpool.tile([P, E_pad], F32, tag="yt_d")
        segstage = dpool.tile([ZR_ROWS, P], F32, tag="segstage")
        ar1_in = dpool.tile([2, P], F32, tag="ar1_in")
        ar1_out = dpool.tile([2, P], F32, tag="ar1_out", addr_space="Shared")
        ar2_in = dpool.tile([2, P], F32, tag="ar2_in")
        ar2_out = dpool.tile([2, P], F32, tag="ar2_out", addr_space="Shared")

        # ---- constants ----
        ws = cpool.tile([P, 13 * P], BF16, tag="ws")
        nc.sync.dma_start(out=ws[:], in_=wsplit[:, :])

        iota_i = cpool.tile([P, P], I32, tag="iota_i")
        nc.gpsimd.iota(iota_i[:], pattern=[[1, P]], base=0, channel_multiplier=0)
        iota = cpool.tile([P, P], BF16, tag="iota")
        nc.vector.tensor_copy(iota[:], iota_i[:])
        idt = cpool.tile([P, P], F32, tag="idt")
        make_identity(nc, idt[:])
        idth = cpool.tile([P, P], BF16, tag="idth")
        nc.vector.tensor_copy(idth[:], idt[:])

        ccol = cpool.tile([P, NC_ROWS], F32, tag="ccol")
        nc.sync.dma_start(out=ccol[:], in_=consts[:, :].rearrange("a b -> b a"))
        crow = cpool.tile([1, NC_ROWS * P], F32, tag="crow")
        nc.sync.dma_start(
            out=crow[:], in_=consts[:, :].rearrange("a b -> (a b)")[None, :])

        def ccol_(i):
            return ccol[:, i:i + 1]

        def crow_(i):
            return crow[:, i * P:(i + 1) * P]

        def wsl(nm):
            o = w_off[nm]
            return ws[:, o:o + P]

        # zero block of segstage
        zt = cpool.tile([P, P], F32, tag="zt")
        nc.vector.memset(zt[:], 0.0)
        nc.sync.dma_start(out=segstage[n_chunks * P:ZR_ROWS, :], in_=zt[:])

        # stats accumulators
        esum = cpool.tile([P, n_chunks], F32, tag="esum")
        esq = cpool.tile([P, n_chunks], F32, tag="esq")
        vsum = cpool.tile([P, NV], F32, tag="vsum")
        vsq = cpool.tile([P, NV], F32, tag="vsq")

        # ================= phase E1 =================
        for t in range(n_chunks):
            esl = slice(t * CHUNK, (t + 1) * CHUNK)

            eftb = sb.tile([P, CHUNK], BF16, tag="eftb")
            nc.sync.dma_start(out=eftb[:], in_=eft_h[:, esl])
            wint = sb.tile([P, P], BF16, tag="wint")
            nc.sync.dma_start(out=wint[:], in_=wins[:, t * P:(t + 1) * P])
            i2t = sb.tile([P, SUB], I32, tag="i2t")
            nc.sync.dma_start(out=i2t[:], in_=i2p[t * P:(t + 1) * P, :])
            i1c = sb.tile([P, SUB], BF16, tag="i1c")
            nc.sync.dma_start(
                out=i1c[:],
                in_=i1rel[esl].rearrange("(a b) -> b a", b=P))

            g2 = sb.tile([P, CHUNK], BF16, tag="g2")
            for j in range(SUB):
                nc.gpsimd.indirect_dma_start(
                    out=g2[:, j * P:(j + 1) * P], out_offset=None,
                    in_=nfpad[:, :],
                    in_offset=IndirectOffsetOnAxis(ap=i2t[:, j:j + 1], axis=0))

            # one-hot (edge-major) and its transpose
            ohe = sb.tile([P, CHUNK], BF16, tag="ohe")
            for j in range(SUB):
                nc.vector.tensor_tensor(
                    out=ohe[:, j * P:(j + 1) * P],
                    in0=i1c[:, j:j + 1].to_broadcast([P, P]),
                    in1=iota[:], op=ALU.is_equal)
            ohn_ps = psT.tile([P, CHUNK], BF16, space="PSUM", tag="t")
            for j in range(SUB):
                nc.tensor.transpose(
                    out=ohn_ps[:, j * P:(j + 1) * P],
                    in_=ohe[:, j * P:(j + 1) * P], identity=idth[:])
            ohn = sb.tile([P, CHUNK], BF16, tag="ohn")
            nc.any.tensor_copy(ohn[:], ohn_ps[:])

            # gathered idx2 rows -> feature-major
            n2ps = psT.tile([P, CHUNK], BF16, space="PSUM", tag="t")
            for j in range(SUB):
                nc.tensor.transpose(
                    out=n2ps[:, j * P:(j + 1) * P],
                    in_=g2[:, j * P:(j + 1) * P], identity=idth[:])
            n2t = sb.tile([P, CHUNK], BF16, tag="n2t")
            nc.any.tensor_copy(n2t[:], n2ps[:])

            # window projection:  PW = window @ We1a   ([n x h])
            pwps = psS.tile([P, CHUNK], F32, space="PSUM", tag="s")
            nc.tensor.matmul(pwps[:, 0:P], lhsT=(wint[:]), rhs=(wsl("e1a")),
                             start=True, stop=True)
            pw = sb.tile([P, P], BF16, tag="pw")
            nc.any.tensor_copy(pw[:], pwps[:, 0:P])

            a1ps = psM.tile([P, CHUNK], F32, space="PSUM", tag="m")
            nc.tensor.matmul(a1ps[:], lhsT=(pw[:]), rhs=(ohn[:]),
                             start=True, stop=False)
            nc.tensor.matmul(a1ps[:], lhsT=(wsl("e1b")), rhs=(n2t[:]),
                             start=False, stop=False)
            nc.tensor.matmul(a1ps[:], lhsT=(wsl("e1c")), rhs=(eftb[:]),
                             start=False, stop=True)
            a1 = sb.tile([P, CHUNK], BF16, tag="a1")
            nc.scalar.activation(a1[:], a1ps[:], AF.Identity, bias=ccol_(C_BE1))
            r1 = sb.tile([P, CHUNK], BF16, tag="r1")
            nc.scalar.activation(r1[:], a1ps[:], AF.Relu, bias=ccol_(C_BE1))

            a2ps = psM.tile([P, CHUNK], F32, space="PSUM", tag="m")
            nc.tensor.matmul(a2ps[:], lhsT=(wsl("e2a")), rhs=(a1[:]),
                             start=True, stop=False)
            nc.tensor.matmul(a2ps[:], lhsT=(wsl("e2r")), rhs=(r1[:]),
                             start=False, stop=True)
            a2 = sb.tile([P, CHUNK], BF16, tag="a2")
            nc.scalar.activation(a2[:], a2ps[:], AF.Identity, bias=ccol_(C_BE2))
            r2 = sb.tile([P, CHUNK], BF16, tag="r2")
            nc.scalar.activation(r2[:], a2ps[:], AF.Relu, bias=ccol_(C_BE2))

            ytps = psM.tile([P, CHUNK], F32, space="PSUM", tag="m")
            nc.tensor.matmul(ytps[:], lhsT=(wsl("e3a")), rhs=(a2[:]),
                             start=True, stop=False)
            nc.tensor.matmul(ytps[:], lhsT=(wsl("e3r")), rhs=(r2[:]),
                             start=False, stop=True)
            ytb = sb.tile([P, CHUNK], F32, tag="ytb")
            nc.scalar.activation(ytb[:], ytps[:], AF.Identity,
                                 bias=ccol_(C_BE3),
                                 accum_out=esum[:, t:t + 1])
            sqd = sb.tile([P, CHUNK], F32, tag="sqd")
            nc.scalar.activation(sqd[:], ytps[:], AF.Square,
                                 bias=ccol_(C_BE3),
                                 accum_out=esq[:, t:t + 1])
            nc.sync.dma_start(out=yt_d[:, esl], in_=ytb[:])

            # edge-major y and segment sums
            yeps = psT.tile([P, CHUNK], F32, space="PSUM", tag="t")
            for j in range(SUB):
                nc.tensor.transpose(
                    out=yeps[:, j * P:(j + 1) * P],
                    in_=ytb[:, j * P:(j + 1) * P], identity=idt[:])
            ye = sb.tile([P, CHUNK], BF16, tag="ye")
            nc.any.tensor_copy(ye[:], yeps[:])

            segps = psS.tile([P, CHUNK], F32, space="PSUM", tag="s")
            for j in range(SUB):
                nc.tensor.matmul(
                    segps[:, 0:P], lhsT=(ohe[:, j * P:(j + 1) * P]),
                    rhs=(ye[:, j * P:(j + 1) * P]),
                    start=(j == 0), stop=(j == SUB - 1))
            segsb = sb.tile([P, P], F32, tag="segsb")
            nc.any.tensor_copy(segsb[:], segps[:, 0:P])
            nc.sync.dma_start(out=segstage[t * P:(t + 1) * P, :], in_=segsb[:])

        # ================= AllReduce #1 (edge BN stats) =================
        st1 = cpool.tile([P, 2], F32, tag="st1")
        nc.vector.reduce_sum(st1[:, 0:1], esum[:], axis=AX.X)
        nc.vector.reduce_sum(st1[:, 1:2], esq[:], axis=AX.X)
        nc.sync.dma_start(out=ar1_in[:, :].rearrange("a b -> b a"), in_=st1[:])
        if dbg_taps:
            nc.sync.dma_start(out=d_st1[:, :], in_=st1[:])
        nc.gpsimd.collective_compute(
            "AllReduce", ALU.add, replica_groups=rg,
            ins=[ar1_in[:, :]], outs=[ar1_out[:, :]])

        def bn_finalize(ar_out, c_scorr, c_qcorr, c_inv, c_g, c_beta, tagp):
            """per-channel s,t columns from all-reduced sum/sumsq"""
            sc = cpool.tile([P, 2], F32, tag=tagp + "sc")
            nc.sync.dma_start(out=sc[:],
                              in_=ar_out[:, :].rearrange("a b -> b a"))
            w = cpool.tile([P, 8], F32, tag=tagp + "w")
            # m = (sum - scorr) * inv
            nc.vector.tensor_tensor(w[:, 0:1], sc[:, 0:1], ccol_(c_scorr),
                                    op=ALU.subtract)
            nc.vector.tensor_tensor(w[:, 0:1], w[:, 0:1], ccol_(c_inv),
                                    op=ALU.mult)
            # q = (sumsq - qcorr) * inv
            nc.vector.tensor_tensor(w[:, 1:2], sc[:, 1:2], ccol_(c_qcorr),
                                    op=ALU.subtract)
            nc.vector.tensor_tensor(w[:, 1:2], w[:, 1:2], ccol_(c_inv),
                                    op=ALU.mult)
            # var = q - m*m ; std = sqrt(var+eps); r = 1/std
            nc.vector.tensor_tensor(w[:, 2:3], w[:, 0:1], w[:, 0:1],
                                    op=ALU.mult)
            nc.vector.tensor_tensor(w[:, 2:3], w[:, 1:2], w[:, 2:3],
                                    op=ALU.subtract)
            nc.vector.tensor_tensor(w[:, 2:3], w[:, 2:3], ccol_(C_EPS),
                                    op=ALU.add)
            nc.scalar.activation(w[:, 3:4], w[:, 2:3], AF.Sqrt)
            nc.vector.reciprocal(w[:, 4:5], w[:, 3:4])
            # s = r * gamma ; t = beta - m*s
            nc.vector.tensor_tensor(w[:, 5:6], w[:, 4:5], ccol_(c_g),
                                    op=ALU.mult)
            nc.vector.tensor_tensor(w[:, 6:7], w[:, 0:1], w[:, 5:6],
                                    op=ALU.mult)
            nc.vector.tensor_tensor(w[:, 7:8], ccol_(c_beta), w[:, 6:7],
                                    op=ALU.subtract)
            return w[:, 5:6], w[:, 7:8]  # s_col, t_col

        se_col, te_col = bn_finalize(ar1_out, C_SCORR_E, C_QCORR_E, C_INVE,
                                     C_GE, C_BETAE, "e")
        if dbg_taps:
            tmpa = cpool.tile([2, P], F32, tag="tmpa")
            nc.sync.dma_start(out=tmpa[:], in_=ar1_out[:])
            nc.sync.dma_start(out=d_ar1[:, :], in_=tmpa[:])
            sec = cpool.tile([P, 2], F32, tag="sec")
            nc.vector.tensor_copy(sec[:, 0:1], se_col)
            nc.vector.tensor_copy(sec[:, 1:2], te_col)
            nc.sync.dma_start(out=d_secol[:, :], in_=sec[:])
            tmpy = cpool.tile([P, CHUNK], F32, tag="tmpy")
            for _t in range(n_chunks):
                _sl = slice(_t * CHUNK, (_t + 1) * CHUNK)
                nc.sync.dma_start(out=tmpy[:], in_=yt_d[:, _sl])
                nc.sync.dma_start(out=d_yt[:, _sl], in_=tmpy[:])
            tmps2 = cpool.tile([P, P], F32, tag="tmps2")
            for _t in range(n_chunks + 1):
                nc.sync.dma_start(out=tmps2[:],
                                  in_=segstage[_t * P:(_t + 1) * P, :])
                nc.sync.dma_start(out=d_seg[_t * P:(_t + 1) * P, :],
                                  in_=tmps2[:])

        # t_e as a row (for the t x mask outer product): tiny PE transpose
        terow_ps = psS.tile([P, CHUNK], F32, space="PSUM", tag="s")
        nc.tensor.transpose(out=terow_ps[0:1, 0:P],
                            in_=te_col, identity=idt[:])
        terow = cpool.tile([1, P], F32, tag="terow")
        nc.any.tensor_copy(terow[:], terow_ps[0:1, 0:P])

        # ================= phase E2 (edge outputs) =================
        for t in range(n_chunks):
            esl = slice(t * CHUNK, (t + 1) * CHUNK)
            ytb2 = sb.tile([P, CHUNK], F32, tag="ytb2")
            nc.sync.dma_start(out=ytb2[:], in_=yt_d[:, esl])
            eftb2 = sb.tile([P, CHUNK], F32, tag="eftb2")
            nc.sync.dma_start(out=eftb2[:], in_=eft[:, esl])
            aff = sb.tile([P, CHUNK], F32, tag="aff")
            nc.scalar.activation(aff[:], ytb2[:], AF.Identity,
                                 bias=te_col, scale=se_col)
            en = sb.tile([P, CHUNK], F32, tag="en")
            nc.vector.tensor_tensor(en[:], aff[:], eftb2[:], op=ALU.add)
            nc.sync.dma_start(out=edge_newt[:, esl], in_=en[:])

        # ================= phase V (nodes) =================
        y2all = cpool.tile([P, N_pad], F32, tag="y2all")
        for v in range(NV):
            vsl = slice(v * CHUNK, (v + 1) * CHUNK)

            vips = psM.tile([P, CHUNK], F32, space="PSUM", tag="m")
            for q in range(SUB):
                i = v * SUB + q
                gx = sb.tile([P, nwin], I32, tag="gx")
                nc.sync.dma_start(out=gx[:], in_=gidx[i * P:(i + 1) * P, :])
                gseg = sb.tile([P, nwin * P], F32, tag="gseg")
                for sdx in range(nwin):
                    nc.gpsimd.indirect_dma_start(
                        out=gseg[:, sdx * P:(sdx + 1) * P], out_offset=None,
                        in_=segstage[:, :],
                        in_offset=IndirectOffsetOnAxis(ap=gx[:, sdx:sdx + 1],
                                                       axis=0))
                accq = sb.tile([P, P], F32, tag="accq")
                if nwin == 1:
                    nc.vector.tensor_copy(accq[:], gseg[:, 0:P])
                else:
                    nc.vector.tensor_tensor(accq[:], gseg[:, 0:P],
                                            gseg[:, P:2 * P], op=ALU.add)
                    for sdx in range(2, nwin):
                        nc.vector.tensor_tensor(
                            accq[:], accq[:],
                            gseg[:, sdx * P:(sdx + 1) * P], op=ALU.add)
                # scaled-diagonal transpose: columns scaled by acol
                ac = sb.tile([P, 1], F32, tag="ac")
                nc.sync.dma_start(
                    out=ac[:], in_=acol[i * P:(i + 1) * P][:, None])
                acs = sb.tile([P, P], F32, tag="acs")
                nc.vector.tensor_scalar(out=acs[:], in0=accq[:],
                                        scalar1=ac[:, 0:1], scalar2=None,
                                        op0=ALU.mult)
                nc.tensor.transpose(out=vips[:, q * P:(q + 1) * P],
                                    in_=acs[:], identity=idt[:])

            vis = sb2.tile([P, CHUNK], F32, tag="vis")
            nc.scalar.activation(vis[:], vips[:], AF.Identity, scale=se_col)
            mr = sb.tile([1, CHUNK], F32, tag="mr")
            nc.sync.dma_start(out=mr[:], in_=mrow[vsl][None, :])
            tmps = psS.tile([P, CHUNK], F32, space="PSUM", tag="s")
            nc.tensor.matmul(tmps[:], lhsT=(terow[:]), rhs=(mr[:]),
                             start=True, stop=True)
            z2a = sb2.tile([P, CHUNK], BF16, tag="z2a")
            nc.vector.tensor_tensor(z2a[:], vis[:], tmps[:], op=ALU.add)
            if dbg_taps:
                z2f = sb2.tile([P, CHUNK], F32, tag="z2f")
                nc.vector.tensor_tensor(z2f[:], vis[:], tmps[:], op=ALU.add)
                nc.sync.dma_start(out=d_vi[:, vsl], in_=z2f[:])

            nfb = sb2.tile([P, CHUNK], BF16, tag="nfb")
            nc.sync.dma_start(out=nfb[:], in_=nftc_h[:, vsl])

            b1ps = psM.tile([P, CHUNK], F32, space="PSUM", tag="m")
            nc.tensor.matmul(b1ps[:], lhsT=(wsl("v1a")), rhs=(z2a[:]),
                             start=True, stop=False)
            nc.tensor.matmul(b1ps[:], lhsT=(wsl("v1b")), rhs=(nfb[:]),
                             start=False, stop=True)
            va1 = sb.tile([P, CHUNK], BF16, tag="va1")
            nc.scalar.activation(va1[:], b1ps[:], AF.Identity, bias=ccol_(C_BV1))
            vr1 = sb.tile([P, CHUNK], BF16, tag="vr1")
            nc.scalar.activation(vr1[:], b1ps[:], AF.Relu, bias=ccol_(C_BV1))

            b2ps = psM.tile([P, CHUNK], F32, space="PSUM", tag="m")
            nc.tensor.matmul(b2ps[:], lhsT=(wsl("v2a")), rhs=(va1[:]),
                             start=True, stop=False)
            nc.tensor.matmul(b2ps[:], lhsT=(wsl("v2r")), rhs=(vr1[:]),
                             start=False, stop=True)
            va2 = sb.tile([P, CHUNK], BF16, tag="va2")
            nc.scalar.activation(va2[:], b2ps[:], AF.Identity, bias=ccol_(C_BV2))
            vr2 = sb.tile([P, CHUNK], BF16, tag="vr2")
            nc.scalar.activation(vr2[:], b2ps[:], AF.Relu, bias=ccol_(C_BV2))

            y2ps = psM.tile([P, CHUNK], F32, space="PSUM", tag="m")
            nc.tensor.matmul(y2ps[:], lhsT=(wsl("v3a")), rhs=(va2[:]),
                             start=True, stop=False)
            nc.tensor.matmul(y2ps[:], lhsT=(wsl("v3r")), rhs=(vr2[:]),
                             start=False, stop=True)
            nc.scalar.activation(y2all[:, vsl], y2ps[:], AF.Identity,
                                 bias=ccol_(C_BV3),
                                 accum_out=vsum[:, v:v + 1])
            sqd2 = sb.tile([P, CHUNK], F32, tag="sqd2")
            nc.scalar.activation(sqd2[:], y2ps[:], AF.Square,
                                 bias=ccol_(C_BV3),
                                 accum_out=vsq[:, v:v + 1])
            if dbg_taps:
                nc.sync.dma_start(out=d_y2[:, vsl], in_=y2all[:, vsl])

        # ================= AllReduce #2 (node BN stats) =================
        st2 = cpool.tile([P, 2], F32, tag="st2")
        nc.vector.reduce_sum(st2[:, 0:1], vsum[:], axis=AX.X)
        nc.vector.reduce_sum(st2[:, 1:2], vsq[:], axis=AX.X)
        nc.sync.dma_start(out=ar2_in[:, :].rearrange("a b -> b a"), in_=st2[:])
        nc.gpsimd.collective_compute(
            "AllReduce", ALU.add, replica_groups=rg,
            ins=[ar2_in[:, :]], outs=[ar2_out[:, :]])
        sv_col, tv_col = bn_finalize(ar2_out, C_SCORR_V, C_QCORR_V, C_INVN,
                                     C_GV, C_BETAV, "v")

        for v in range(NV):
            vsl = slice(v * CHUNK, (v + 1) * CHUNK)
            affv = sb.tile([P, CHUNK], F32, tag="affv")
            nc.scalar.activation(affv[:], y2all[:, vsl], AF.Identity,
                                 bias=tv_col, scale=sv_col)
            nfb2 = sb.tile([P, CHUNK], F32, tag="nfb2")
            nc.sync.dma_start(out=nfb2[:], in_=nftc[:, vsl])
            nn = sb.tile([P, CHUNK], F32, tag="nn")
            nc.vector.tensor_tensor(nn[:], affv[:], nfb2[:], op=ALU.add)
            nc.sync.dma_start(out=node_newt[:, vsl], in_=nn[:])

    nc.compile()
    return nc


# --------------------------------------------------------------------------
# entry point
# --------------------------------------------------------------------------

LAST_RESULT = None


def _ensure_ntff_hook():
    """Install the NTFF profile hook when the image's antenv lacks it."""
    import sys
    import types
    try:
        from antenv.axon_hooks import get_axon_ntff_profile_hook  # noqa: F401
        return
    except ImportError:
        pass
    try:
        import antenv
        from trn_agent_boot.trn_boot import _ntff_profile_via_ctypes
        mod = types.ModuleType("antenv.axon_hooks")
        mod._hook = _ntff_profile_via_ctypes("/opt/axon/libaxon_pjrt.so")
        mod.get_axon_ntff_profile_hook = lambda: mod._hook
        mod.set_axon_ntff_profile_hook = lambda h: setattr(mod, "_hook", h)
        sys.modules["antenv.axon_hooks"] = mod
        antenv.axon_hooks = mod
    except Exception as e:  # profiling is best-effort
        print(f"ntff hook install failed: {e}")



def kernel(node_fea, edge_fea, idx1, idx2,
           We1, be1, We2, be2, We3, be3,
           Wv1, bv1, Wv2, bv2, Wv3, bv3,
           gamma_e, beta_e, gamma_v, beta_v):
    node_fea = np.asarray(node_fea, np.float32)
    edge_fea = np.asarray(edge_fea, np.float32)
    idx1 = np.asarray(idx1).astype(np.int64).ravel()
    idx2 = np.asarray(idx2).astype(np.int64).ravel()
    W = dict(We1=np.asarray(We1, np.float32), be1=np.asarray(be1, np.float32),
             We2=np.asarray(We2, np.float32), be2=np.asarray(be2, np.float32),
             We3=np.asarray(We3, np.float32), be3=np.asarray(be3, np.float32),
             Wv1=np.asarray(Wv1, np.float32), bv1=np.asarray(bv1, np.float32),
             Wv2=np.asarray(Wv2, np.float32), bv2=np.asarray(bv2, np.float32),
             Wv3=np.asarray(Wv3, np.float32), bv3=np.asarray(bv3, np.float32),
             gamma_e=np.asarray(gamma_e, np.float32),
             beta_e=np.asarray(beta_e, np.float32),
             gamma_v=np.asarray(gamma_v, np.float32),
             beta_v=np.asarray(beta_v, np.float32))

    cfg = _host_prep(node_fea, edge_fea, idx1, idx2, W)
    nc = build_program(cfg.N, cfg.n_chunks, cfg.N_pad, cfg.nwin, cfg.w_off)
    trace = os.environ.get("BASS_KERNEL_TRACE") == "1"
    if trace:
        _ensure_ntff_hook()
    global LAST_RESULT
    LAST_RESULT = run_bass_kernel_spmd(nc, cfg.in_maps, list(range(NCORES)),
                                       trace=trace)
    res = LAST_RESULT.results

    N, E = cfg.N, cfg.E
    node_new = np.zeros((N, P), np.float32)
    edge_new = np.zeros((E, P), np.float32)
    for k, c in enumerate(cfg.cores):
        nnT = res[k]["node_newt"]
        enT = res[k]["edge_newt"]
        node_new[c["n0"]:c["n1"]] = nnT[:, :c["N_loc"]].T
        valid = c["eid"] >= 0
        edge_new[c["eid"][valid]] = enT[:, valid].T
    return node_new, edge_new
